# revision 40
# baseline (speedup 1.0000x reference)
"""Block-diagonal causal GQA attention with RoPE, sharded over 8 TRN2 cores.

Problem (hardcoded from the spec):
  x [4096, 4096], wq [4096, 4096] (32 q heads x 128), wk/wv [4096, 1024]
  (8 kv heads), wo [4096, 4096], freqs_cos/sin [4096, 64], block_size 1024.
  4 independent causal blocks of 1024 tokens.

Sharding: 8 cores = 4 sequence blocks x 2 head-groups.  Core (b, g)
computes block b for q-heads [16g, 16g+16) (kv heads [4g, 4g+4)) and the
partial output projection through the matching rows of wo.  The host sums
the two head-group partials per block and concatenates the blocks.

Device layout notes:
  - x block is fed pre-transposed (xbT [4096, 1024]) so the DIM contraction
    sits on SBUF partitions for the QKV projections.
  - wq/wk columns are de-interleaved per head on the host (even head-dims
    then odd head-dims) so RoPE's pair rotation becomes a [0:64]/[64:128]
    partition-half operation on the Q^T/K^T layout.
  - attention runs in the transposed-score layout S^T [j, i]: scores come
    out of the PE as S^T tiles, the softmax denominator is a ones-vector
    matmul, and P^T feeds the PV and WO matmuls directly (no transposes).
"""

import numpy as np
from contextlib import ExitStack

import concourse.bass as bass
import concourse.tile as tile
import concourse.mybir as mybir
from concourse import bass_utils
import concourse.tile_sem_assignment as _tsa

F32 = mybir.dt.float32

# -- full-problem constants ---------------------------------------------------
DIM = 4096
BLOCK = 1024
D = 128            # head dim
HQ = 16            # q heads per core
HKV = 4            # kv heads per core
N_CORES = 8
NEG = -1.0e9
# matmul compute dtype: float32 (4 cyc/row, accurate) or float32r (1 cyc/row)
import os as _os
MM_DT = (mybir.dt.float32r if _os.environ.get("MM_DT", "f32r") == "f32r"
         else mybir.dt.float32)


def _trim_dma_waits(nc):
    """Drop DMA semaphore waits that are transitively guaranteed.

    The DGE descriptor path supports only 2 sync-wait commands per DMA,
    but Tile's wait emission is not transitively minimal.  We compute,
    for every instruction, a conservative "floor": the semaphore values
    guaranteed to have been reached by the time it completes (its own
    waits, the floors of the instructions those waits observe, the
    floors of its sync dependencies, plus in-order completion along each
    semaphore's single FIFO ring).  A wait on a DMA is dead if the
    floors implied by its remaining waits already cover it.
    """
    import bass_rust

    insts = []
    for blk in nc.m.functions[0].blocks:
        insts.extend(blk.instructions)

    floors: dict[str, dict[int, int]] = {}     # inst name -> {sem id: value}
    chain: dict[int, list[tuple[int, str]]] = {}  # sem id -> [(post_val, name)]
    cum: dict[int, int] = {}

    def sem_floor(sem_id, v):
        """Floor implied by observing sem_id >= v (completion of the
        instruction whose update reached v, FIFO within a sem)."""
        lst = chain.get(sem_id)
        if not lst:
            return None
        # smallest post_val >= v
        import bisect
        idx = bisect.bisect_left(lst, (v, ""))
        if idx == len(lst):
            return None
        return floors.get(lst[idx][1])

    def merge(dst, src):
        if not src:
            return
        for k, v in src.items():
            if dst.get(k, -1) < v:
                dst[k] = v

    for ins in insts:
        si = ins.sync_info
        fl: dict[int, int] = {}
        if si is not None:
            for w in si.on_wait:
                if w.wait_mode != "sem-ge-imm" or w.wait_value is None:
                    continue
                if fl.get(w.id, -1) < w.wait_value:
                    fl[w.id] = w.wait_value
                merge(fl, sem_floor(w.id, w.wait_value))
        try:
            for dn in ins.sync_dependency_names():
                merge(fl, floors.get(dn))
        except TypeError:
            pass
        if si is not None:
            for u in si.on_update:
                if u.update_mode not in ("sem-add-imm", "sem-inc") \
                        or u.update_value is None:
                    continue
                post = cum.get(u.id, 0) + u.update_value
                cum[u.id] = post
                lst = chain.setdefault(u.id, [])
                # in-order completion per sem ring: inherit previous floor
                if lst:
                    merge(fl, floors.get(lst[-1][1]))
                if fl.get(u.id, -1) < post:
                    fl[u.id] = post
                lst.append((post, ins.name))
        floors[ins.name] = fl

    leftover = []
    for ins in insts:
        if not isinstance(ins, mybir.InstDMACopy):
            continue
        si = ins.sync_info
        if si is None:
            continue
        waits = list(si.on_wait)
        changed = True
        while len(waits) > 1 and changed:
            changed = False
            for i, w in enumerate(waits):
                if w.wait_mode != "sem-ge-imm" or w.wait_value is None:
                    continue
                implied: dict[int, int] = {}
                for j, w2 in enumerate(waits):
                    if j == i or w2.wait_mode != "sem-ge-imm":
                        continue
                    merge(implied, sem_floor(w2.id, w2.wait_value))
                if implied.get(w.id, -1) >= w.wait_value:
                    waits.pop(i)
                    changed = True
                    break
        if len(waits) != len(si.on_wait):
            ins.sync_info = bass_rust.SyncInfo(
                on_wait=waits, on_update=list(si.on_update))
    del leftover  # remaining multi-waits are split into engine prefixes


def _split_waits_json(bir):
    """Split multi-wait instructions at the BIR level.

    walrus' setupSyncWait budget: one wait of any value, or two waits
    whose values both fit a one-byte command.  Excess waits move onto
    standalone EventSemaphore instructions inserted directly before the
    instruction on the same engine — engines execute their stream in
    order, so a prefix wait is semantically identical to an attached
    one.  (DMAs are enqueued by their issuing engine in stream order,
    so the same argument holds for the enqueue.)
    """
    nid = 0
    for fn in bir["functions"]:
        for blk in fn["blocks"]:
            out = []
            for ins in blk["instructions"]:
                si = ins.get("sync_info")
                waits = (si or {}).get("on_wait") or []
                if len(waits) > 1:
                    waits = sorted(
                        waits, key=lambda w: -(w.get("wait_value") or 0))
                    for w in waits[1:]:
                        nid += 1
                        out.append({
                            "debug": ins.get("debug"),
                            "engine": ins["engine"],
                            "ins": [],
                            "outs": [],
                            "name": f"{ins['name']}-w{nid}",
                            "opcode": "EventSemaphore",
                            "sync_info": {"on_update": [], "on_wait": [w]},
                        })
                    si["on_wait"] = waits[:1]
                out.append(ins)
            blk["instructions"] = out
    return bir


def build_kernel(dim=DIM, block=BLOCK, hq=HQ, hkv=HKV, mm_dt=MM_DT):
    """Emit the per-core Bass program (SPMD: same program on all cores)."""
    rep = hq // hkv
    KC = dim // 128          # contraction chunks
    assert KC % 2 == 0
    KH = KC // 2             # chunks per half
    W = min(512, block)      # psum free width
    CH = block // W          # i-chunks per block
    NI = block // 128        # j-tiles per block
    assert NI <= 8
    HSET = max(1, 8 // CH)   # q heads per psum-set
    HALF_D = D // 2
    SCALE = float(1.0 / np.sqrt(D))
    assert hkv * D <= 512

    nc = bass.Bass("TRN2", target_bir_lowering=False, debug=False)

    xbT = nc.dram_tensor("xbT", [dim, block], F32, kind="ExternalInput").ap()
    wq = nc.dram_tensor("wq", [dim, hq * D], F32, kind="ExternalInput").ap()
    wk = nc.dram_tensor("wk", [dim, hkv * D], F32, kind="ExternalInput").ap()
    wv = nc.dram_tensor("wv", [dim, hkv * D], F32, kind="ExternalInput").ap()
    wo = nc.dram_tensor("wo", [hq * D, dim], F32, kind="ExternalInput").ap()
    # cos2: [cos; cos] stacked to 128 partitions; sin2: [-sin; +sin]
    cos2 = nc.dram_tensor("cos2", [D, block], F32, kind="ExternalInput").ap()
    sin2 = nc.dram_tensor("sin2", [D, block], F32, kind="ExternalInput").ap()
    out = nc.dram_tensor("out", [block, dim], F32, kind="ExternalOutput").ap()

    MDT = mm_dt  # dtype of all matmul-input SBUF tiles

    def mm(out_ap, lhsT, rhs, **kw):
        nc.tensor.matmul(out_ap, lhsT, rhs, **kw)

    def ld(dst, src):
        # DMA load with dtype relabel for fp32r tiles
        if mm_dt != F32:
            src = src.bitcast(mm_dt)
        nc.sync.dma_start(dst, src)

    # expS free-dim layout: j-tile t occupies [offs[t], offs[t] + block - 128 t)
    offs = []
    o = 0
    for t in range(NI):
        offs.append(o)
        o += block - t * 128
    EW = o

    with tile.TileContext(nc) as tc, ExitStack() as ctx:
        const = ctx.enter_context(tc.tile_pool(name="const", bufs=1))
        ones = const.tile([128, 1], MDT)
        nc.gpsimd.memset(ones[:].bitcast(F32), 1.0)
        ones_row = const.tile([1, 128], MDT)
        nc.gpsimd.memset(ones_row[:].bitcast(F32), 1.0)
        # additive causal mask for the diagonal 128x128 block of S^T
        # (partition x = j_local, free y = i_local; invalid where y < x)
        tri = const.tile([128, 128], F32)
        nc.gpsimd.memset(tri[:], 0.0)
        # keep (0.0) where y - x >= 0 (i.e. i_local >= j_local), else NEG
        nc.gpsimd.affine_select(
            out=tri[:], in_=tri[:],
            compare_op=mybir.AluOpType.is_ge,
            fill=NEG, base=0, pattern=[[1, 128]], channel_multiplier=-1,
        )

        with tc.tile_pool(name="accs", bufs=1) as acc_pool:
            # persistent SBUF accumulators, one big tile each, sliced per head
            qTa = acc_pool.tile([128, hq * block], MDT)     # per head: [d, i]
            kTa = acc_pool.tile([128, hkv * block], MDT)    # per kv head: [d, j]
            va = acc_pool.tile([128, NI * hkv * D], MDT)    # per j-tile: [j, hkv*D]

            # ---- QKV projections, two k-halves so SBUF holds half of xbT.
            # ropep is allocated before xbp/wsp (fresh addresses, so the
            # rope swap DMAs carry no address-reuse waits) and released
            # with them, freeing its space for the attention pools. ----
            with tc.tile_pool(name="ropep", bufs=3) as rp, \
                 tc.tile_pool(name="xbp", bufs=KH) as xb_pool, \
                 tc.tile_pool(name="wsp", bufs=4) as ws_pool, \
                 tc.tile_pool(name="qkvps", bufs=8, space="PSUM") as ps_pool:

                # cos/sin live only until RoPE is done, so they share the
                # QKV-scoped pool space instead of the ctx-lifetime const pool
                cos_sb = rp.tile([D, block], F32, name="cos_sb", tag="cos", bufs=1)
                nc.sync.dma_start(cos_sb[:], cos2)
                sin_sb = rp.tile([D, block], F32, name="sin_sb", tag="sin", bufs=1)
                nc.sync.dma_start(sin_sb[:], sin2)

                def acc_store(dst, ps, half):
                    if half == 0:
                        nc.scalar.copy(dst, ps)
                    else:
                        nc.vector.tensor_add(dst, dst, ps)

                for half in range(2):
                    xb = []
                    for kk in range(KH):
                        k = half * KH + kk
                        xt = xb_pool.tile([128, block], MDT, name="xbt", tag="xb")
                        ld(xt[:], xbT[k * 128:(k + 1) * 128, :])
                        xb.append(xt)

                    # Q^T: per head [d, i] accumulated over k
                    for hs in range(0, hq, HSET):
                        nh = min(HSET, hq - hs)
                        ps = [[ps_pool.tile([128, W], F32, name="qps", tag="ps")
                               for _ in range(CH)] for _ in range(nh)]
                        for kk in range(KH):
                            k = half * KH + kk
                            wt = ws_pool.tile([128, HSET * D], MDT, name="wqs", tag="ws")
                            ld(wt[:, :nh * D],
                               wq[k * 128:(k + 1) * 128, hs * D:(hs + nh) * D])
                            for hl in range(nh):
                                for c in range(CH):
                                    mm(ps[hl][c][:], wt[:, hl * D:(hl + 1) * D],
                                       xb[kk][:, c * W:(c + 1) * W],
                                       start=(kk == 0), stop=(kk == KH - 1))
                        for hl in range(nh):
                            for c in range(CH):
                                h = hs + hl
                                dst = qTa[:, h * block + c * W: h * block + (c + 1) * W]
                                acc_store(dst, ps[hl][c][:], half)

                    # K^T: per kv head [d, j]
                    for hs in range(0, hkv, HSET):
                        nh = min(HSET, hkv - hs)
                        ps = [[ps_pool.tile([128, W], F32, name="kps", tag="ps")
                               for _ in range(CH)] for _ in range(nh)]
                        for kk in range(KH):
                            k = half * KH + kk
                            wt = ws_pool.tile([128, HSET * D], MDT, name="wks", tag="ws")
                            ld(wt[:, :nh * D],
                               wk[k * 128:(k + 1) * 128, hs * D:(hs + nh) * D])
                            for hl in range(nh):
                                for c in range(CH):
                                    mm(ps[hl][c][:], wt[:, hl * D:(hl + 1) * D],
                                       xb[kk][:, c * W:(c + 1) * W],
                                       start=(kk == 0), stop=(kk == KH - 1))
                        for hl in range(nh):
                            for c in range(CH):
                                h = hs + hl
                                dst = kTa[:, h * block + c * W: h * block + (c + 1) * W]
                                acc_store(dst, ps[hl][c][:], half)

                    # V: per j-tile [j, hkv*D]
                    for ts in range(0, NI, 8):
                        nt = min(8, NI - ts)
                        ps = [ps_pool.tile([128, W], F32, name="vps", tag="ps")
                              for _ in range(nt)]
                        for kk in range(KH):
                            k = half * KH + kk
                            wt = ws_pool.tile([128, HSET * D], MDT, name="wvs", tag="ws")
                            ld(wt[:, :hkv * D],
                               wv[k * 128:(k + 1) * 128, :])
                            for tl in range(nt):
                                tj = ts + tl
                                mm(ps[tl][:, :hkv * D],
                                   xb[kk][:, tj * 128:(tj + 1) * 128],
                                   wt[:, :hkv * D],
                                   start=(kk == 0), stop=(kk == KH - 1))
                        for tl in range(nt):
                            tj = ts + tl
                            dst = va[:, tj * hkv * D:(tj + 1) * hkv * D]
                            acc_store(dst, ps[tl][:, :hkv * D], half)

                # ---- RoPE on Q^T / K^T (in SBUF, in place) ----
                # Layout per head: partitions [0:64] = even head-dims,
                # [64:128] = odd.  rope(x) = x*cos2 + swap(x)*sin2 where
                # swap exchanges the halves (SBUF->SBUF DMA) and
                # sin2 = [-sin; +sin], cos2 = [cos; cos].
                def rope(base):
                    sw = rp.tile([D, block], MDT, name="sw", tag="sw")
                    nc.sync.dma_start(sw[0:HALF_D, :], base[HALF_D:D, :])
                    nc.sync.dma_start(sw[HALF_D:D, :], base[0:HALF_D, :])
                    tmp = rp.tile([D, block], F32, name="rtmp", tag="rtmp")
                    nc.vector.tensor_mul(tmp[:], sw[:], sin_sb[:])
                    nc.vector.tensor_mul(base, base, cos_sb[:])
                    nc.vector.tensor_add(base, base, tmp[:])

                for h in range(hq):
                    rope(qTa[:, h * block:(h + 1) * block])
                for h in range(hkv):
                    rope(kTa[:, h * block:(h + 1) * block])

            # O^T persists in SBUF through attention and the output
            # projection (no DRAM roundtrip); allocated after the QKV
            # pools release so it reuses their space, and entered on ctx
            # so it outlives the accs block.
            oT_pool = ctx.enter_context(
                tc.tile_pool(name="oTallp", bufs=1, side="right"))
            oTall = oT_pool.tile([128, hq * block], MDT, name="oTall")

            # ---- attention per q head ----
            with tc.tile_pool(name="attsb", bufs=2) as att_sb, \
                 tc.tile_pool(name="stps", bufs=3, space="PSUM") as st_ps, \
                 tc.tile_pool(name="sumps", bufs=2, space="PSUM") as sum_ps, \
                 tc.tile_pool(name="bcps", bufs=1, space="PSUM") as bc_ps, \
                 tc.tile_pool(name="pvps", bufs=2, space="PSUM") as pv_ps:
                def scores(h):
                    # S^T (causal trapezoid) -> exp
                    kv = h // rep
                    qT = qTa[:, h * block:(h + 1) * block]
                    kT = kTa[:, kv * block:(kv + 1) * block]
                    expS = att_sb.tile([128, EW], MDT, name="expS", tag="expS")
                    for t in range(NI):
                        i0 = t * 128
                        for c in range(CH):
                            s0 = max(i0, c * W)
                            s1 = (c + 1) * W
                            if s0 >= s1:
                                continue
                            w = s1 - s0
                            st = st_ps.tile([128, W], F32, name="st", tag="st")
                            mm(st[:, :w], kT[:, i0:i0 + 128], qT[:, s0:s1],
                               start=True, stop=True)
                            if s0 == i0:  # diagonal block at local [0:128)
                                nc.vector.tensor_add(st[:, 0:128], st[:, 0:128], tri[:])
                            e0 = offs[t] + (s0 - i0)
                            nc.scalar.activation(
                                expS[:, e0:e0 + w], st[:, :w],
                                mybir.ActivationFunctionType.Exp, scale=SCALE)
                    return expS

                def finish(h, expS):
                    # denominator (ones^T @ expS^T -> reciprocal), PV, then a
                    # K=1 outer-product broadcast of 1/sum and the normalize.
                    # PV is emitted before the broadcast so the PE never waits
                    # on the DVE reciprocal.
                    kv = h // rep
                    rcs = []
                    for c in range(CH):
                        live = [t for t in range(NI) if t * 128 < (c + 1) * W]
                        sp = sum_ps.tile([1, W], F32, name="sump", tag="sump")
                        for idx, t in enumerate(live):
                            i0 = t * 128
                            s0 = max(i0, c * W)
                            w = (c + 1) * W - s0
                            e0 = offs[t] + (s0 - i0)
                            mm(sp[:1, s0 - c * W:s0 - c * W + w],
                               ones[:, :1], expS[:, e0:e0 + w],
                               start=(idx == 0), stop=(idx == len(live) - 1))
                        rc = att_sb.tile([1, W], MDT, name="rc", tag="rc")
                        with nc.allow_low_precision("fp32r matmul operand"):
                            nc.vector.reciprocal(rc[:1, :W], sp[:1, :W])
                        rcs.append(rc)

                    pvs = []
                    for c in range(CH):
                        live = [t for t in range(NI) if t * 128 < (c + 1) * W]
                        pv = pv_ps.tile([128, W], F32, name="pv", tag="pv")
                        for idx, t in enumerate(live):
                            i0 = t * 128
                            s0 = max(i0, c * W)
                            w = (c + 1) * W - s0
                            e0 = offs[t] + (s0 - i0)
                            mm(pv[:, s0 - c * W:s0 - c * W + w],
                               va[:, t * hkv * D + kv * D: t * hkv * D + (kv + 1) * D],
                               expS[:, e0:e0 + w],
                               start=(idx == 0), stop=(idx == len(live) - 1))
                        pvs.append(pv)

                    recipB = att_sb.tile([128, block], F32, name="recipB",
                                         tag="recipB", bufs=1)
                    for c in range(CH):
                        bc = bc_ps.tile([128, W], F32, name="bc", tag="bc")
                        mm(bc[:], ones_row[:1, :], rcs[c][:1, :W],
                           start=True, stop=True)
                        nc.scalar.copy(recipB[:, c * W:(c + 1) * W], bc[:])
                        nc.vector.tensor_mul(
                            oTall[:, h * block + c * W: h * block + (c + 1) * W],
                            pvs[c][:, :W],
                            recipB[:, c * W:(c + 1) * W])

                # two-stage pipeline: S^T/exp of head h+1 overlaps the
                # denominator/PV work of head h, so the PE always has
                # independent matmuls while ACT runs exp
                prev = None
                for h in range(hq):
                    e = scores(h)
                    if prev is not None:
                        finish(*prev)
                    prev = (h, e)
                finish(*prev)

        # ---- output projection: out = O @ wo_g ----
        with tc.tile_pool(name="wow", bufs=2 * hq) as wo_pool, \
             tc.tile_pool(name="woout", bufs=4) as out_pool, \
             tc.tile_pool(name="wops", bufs=4, space="PSUM") as wo_ps:
            for nch in range(dim // W):
                wts = []
                for h in range(hq):
                    wt = wo_pool.tile([128, W], MDT, name="wot", tag="wot")
                    ld(wt[:], wo[h * D:(h + 1) * D, nch * W:(nch + 1) * W])
                    wts.append(wt)
                for it in range(NI):
                    ps = wo_ps.tile([128, W], F32, name="wop", tag="wop")
                    for h in range(hq):
                        mm(ps[:], oTall[:, h * block + it * 128: h * block + it * 128 + 128],
                           wts[h][:], start=(h == 0), stop=(h == hq - 1))
                    ob = out_pool.tile([128, W], F32, name="ob", tag="ob")
                    nc.scalar.copy(ob[:], ps[:])
                    nc.sync.dma_start(out[it * 128:(it + 1) * 128, nch * W:(nch + 1) * W],
                                      ob[:])
    _trim_dma_waits(nc)
    import json as _json
    _fixed = _json.dumps(_split_waits_json(
        _json.loads(nc.to_json_bytes()))).encode()
    nc.to_json_bytes = lambda: _fixed
    return nc


def _deinterleave_cols(w, nheads):
    """Per head, reorder the 128 columns to [even head-dims, odd head-dims]."""
    dim = w.shape[0]
    r = w.reshape(dim, nheads, D // 2, 2)
    return np.concatenate([r[..., 0], r[..., 1]], axis=2).reshape(dim, nheads * D)


def shard_inputs(x, wq, wk, wv, wo, freqs_cos, freqs_sin):
    """Build the 8 per-core input maps (core = 2*block + head_group)."""
    x = np.ascontiguousarray(np.asarray(x, dtype=np.float32))
    wq_p = _deinterleave_cols(np.asarray(wq, dtype=np.float32), 32)
    wk_p = _deinterleave_cols(np.asarray(wk, dtype=np.float32), 8)
    wv = np.asarray(wv, dtype=np.float32)
    wo = np.asarray(wo, dtype=np.float32)
    cos = np.asarray(freqs_cos, dtype=np.float32)
    sin = np.asarray(freqs_sin, dtype=np.float32)

    wq_h = wq_p.reshape(DIM, 32, D)
    wk_h = wk_p.reshape(DIM, 8, D)
    wv_h = wv.reshape(DIM, 8, D)
    wo_h = wo.reshape(32, D, DIM)

    in_maps = []
    for core in range(N_CORES):
        b, g = divmod(core, 2)
        rows = slice(b * BLOCK, (b + 1) * BLOCK)
        cosT = cos[rows].T                       # [64, block]
        sinT = sin[rows].T
        cos2 = np.concatenate([cosT, cosT], axis=0)     # [128, block]
        sin2 = np.concatenate([-sinT, sinT], axis=0)
        in_maps.append({
            "xbT": np.ascontiguousarray(x[rows, :].T),
            "wq": np.ascontiguousarray(
                wq_h[:, g * HQ:(g + 1) * HQ].reshape(DIM, HQ * D)),
            "wk": np.ascontiguousarray(
                wk_h[:, g * HKV:(g + 1) * HKV].reshape(DIM, HKV * D)),
            "wv": np.ascontiguousarray(
                wv_h[:, g * HKV:(g + 1) * HKV].reshape(DIM, HKV * D)),
            "wo": np.ascontiguousarray(
                wo_h[g * HQ:(g + 1) * HQ].reshape(HQ * D, DIM)),
            "cos2": np.ascontiguousarray(cos2),
            "sin2": np.ascontiguousarray(sin2),
        })
    return in_maps


def unshard_output(core_outs):
    full = np.empty((NB_TOTAL, DIM), dtype=np.float32)
    for b in range(NB_TOTAL // BLOCK):
        full[b * BLOCK:(b + 1) * BLOCK] = core_outs[2 * b] + core_outs[2 * b + 1]
    return full


NB_TOTAL = 4096  # total sequence length

_NC_CACHE = {}


def _get_nc():
    key = (DIM, BLOCK, HQ, HKV, str(MM_DT))
    if key not in _NC_CACHE:
        _NC_CACHE[key] = build_kernel()
    return _NC_CACHE[key]


def kernel(x, wq, wk, wv, wo, freqs_cos, freqs_sin, block_size, **run_kwargs):
    assert int(block_size) == BLOCK, f"unexpected block_size {block_size}"
    in_maps = shard_inputs(x, wq, wk, wv, wo, freqs_cos, freqs_sin)
    nc = _get_nc()
    res = bass_utils.run_bass_kernel_spmd(
        nc, in_maps, core_ids=list(range(N_CORES)), **run_kwargs)
    outs = [r["out"] for r in res.results]
    out = unshard_output(outs)
    kernel.last_results = res
    return out


# revision 41
# speedup vs baseline: 1.0262x; 1.0262x over previous
"""Block-diagonal causal GQA attention with RoPE, sharded over 8 TRN2 cores.

Problem (hardcoded from the spec):
  x [4096, 4096], wq [4096, 4096] (32 q heads x 128), wk/wv [4096, 1024]
  (8 kv heads), wo [4096, 4096], freqs_cos/sin [4096, 64], block_size 1024.
  4 independent causal blocks of 1024 tokens.

Sharding: 8 cores = 4 sequence blocks x 2 head-groups.  Core (b, g)
computes block b for q-heads [16g, 16g+16) (kv heads [4g, 4g+4)) and the
partial output projection through the matching rows of wo.  The host sums
the two head-group partials per block and concatenates the blocks.

Device layout notes:
  - x block is fed pre-transposed (xbT [4096, 1024]) so the DIM contraction
    sits on SBUF partitions for the QKV projections.
  - wq/wk columns are de-interleaved per head on the host (even head-dims
    then odd head-dims) so RoPE's pair rotation becomes a [0:64]/[64:128]
    partition-half operation on the Q^T/K^T layout.
  - attention runs in the transposed-score layout S^T [j, i]: scores come
    out of the PE as S^T tiles, the softmax denominator is a ones-vector
    matmul, and P^T feeds the PV and WO matmuls directly (no transposes).
"""

import numpy as np
from contextlib import ExitStack

import concourse.bass as bass
import concourse.tile as tile
import concourse.mybir as mybir
from concourse import bass_utils
import concourse.tile_sem_assignment as _tsa

F32 = mybir.dt.float32

# -- full-problem constants ---------------------------------------------------
DIM = 4096
BLOCK = 1024
D = 128            # head dim
HQ = 16            # q heads per core
HKV = 4            # kv heads per core
N_CORES = 8
NEG = -1.0e9
# matmul compute dtype: float32 (4 cyc/row, accurate) or float32r (1 cyc/row)
import os as _os
MM_DT = (mybir.dt.float32r if _os.environ.get("MM_DT", "f32r") == "f32r"
         else mybir.dt.float32)


def _trim_dma_waits(nc):
    """Drop DMA semaphore waits that are transitively guaranteed.

    The DGE descriptor path supports only 2 sync-wait commands per DMA,
    but Tile's wait emission is not transitively minimal.  We compute,
    for every instruction, a conservative "floor": the semaphore values
    guaranteed to have been reached by the time it completes (its own
    waits, the floors of the instructions those waits observe, the
    floors of its sync dependencies, plus in-order completion along each
    semaphore's single FIFO ring).  A wait on a DMA is dead if the
    floors implied by its remaining waits already cover it.
    """
    import bass_rust

    insts = []
    for blk in nc.m.functions[0].blocks:
        insts.extend(blk.instructions)

    floors: dict[str, dict[int, int]] = {}     # inst name -> {sem id: value}
    chain: dict[int, list[tuple[int, str]]] = {}  # sem id -> [(post_val, name)]
    cum: dict[int, int] = {}

    def sem_floor(sem_id, v):
        """Floor implied by observing sem_id >= v (completion of the
        instruction whose update reached v, FIFO within a sem)."""
        lst = chain.get(sem_id)
        if not lst:
            return None
        # smallest post_val >= v
        import bisect
        idx = bisect.bisect_left(lst, (v, ""))
        if idx == len(lst):
            return None
        return floors.get(lst[idx][1])

    def merge(dst, src):
        if not src:
            return
        for k, v in src.items():
            if dst.get(k, -1) < v:
                dst[k] = v

    for ins in insts:
        si = ins.sync_info
        fl: dict[int, int] = {}
        if si is not None:
            for w in si.on_wait:
                if w.wait_mode != "sem-ge-imm" or w.wait_value is None:
                    continue
                if fl.get(w.id, -1) < w.wait_value:
                    fl[w.id] = w.wait_value
                merge(fl, sem_floor(w.id, w.wait_value))
        try:
            for dn in ins.sync_dependency_names():
                merge(fl, floors.get(dn))
        except TypeError:
            pass
        if si is not None:
            for u in si.on_update:
                if u.update_mode not in ("sem-add-imm", "sem-inc") \
                        or u.update_value is None:
                    continue
                post = cum.get(u.id, 0) + u.update_value
                cum[u.id] = post
                lst = chain.setdefault(u.id, [])
                # in-order completion per sem ring: inherit previous floor
                if lst:
                    merge(fl, floors.get(lst[-1][1]))
                if fl.get(u.id, -1) < post:
                    fl[u.id] = post
                lst.append((post, ins.name))
        floors[ins.name] = fl

    leftover = []
    for ins in insts:
        if not isinstance(ins, mybir.InstDMACopy):
            continue
        si = ins.sync_info
        if si is None:
            continue
        waits = list(si.on_wait)
        changed = True
        while len(waits) > 1 and changed:
            changed = False
            for i, w in enumerate(waits):
                if w.wait_mode != "sem-ge-imm" or w.wait_value is None:
                    continue
                implied: dict[int, int] = {}
                for j, w2 in enumerate(waits):
                    if j == i or w2.wait_mode != "sem-ge-imm":
                        continue
                    merge(implied, sem_floor(w2.id, w2.wait_value))
                if implied.get(w.id, -1) >= w.wait_value:
                    waits.pop(i)
                    changed = True
                    break
        if len(waits) != len(si.on_wait):
            ins.sync_info = bass_rust.SyncInfo(
                on_wait=waits, on_update=list(si.on_update))
    del leftover  # remaining multi-waits are split into engine prefixes


def _split_waits_json(bir):
    """Split multi-wait instructions at the BIR level.

    walrus' setupSyncWait budget: one wait of any value, or two waits
    whose values both fit a one-byte command.  Excess waits move onto
    standalone EventSemaphore instructions inserted directly before the
    instruction on the same engine — engines execute their stream in
    order, so a prefix wait is semantically identical to an attached
    one.  (DMAs are enqueued by their issuing engine in stream order,
    so the same argument holds for the enqueue.)
    """
    nid = 0
    for fn in bir["functions"]:
        for blk in fn["blocks"]:
            out = []
            for ins in blk["instructions"]:
                si = ins.get("sync_info")
                waits = (si or {}).get("on_wait") or []
                if len(waits) > 1:
                    waits = sorted(
                        waits, key=lambda w: -(w.get("wait_value") or 0))
                    for w in waits[1:]:
                        nid += 1
                        out.append({
                            "debug": ins.get("debug"),
                            "engine": ins["engine"],
                            "ins": [],
                            "outs": [],
                            "name": f"{ins['name']}-w{nid}",
                            "opcode": "EventSemaphore",
                            "sync_info": {"on_update": [], "on_wait": [w]},
                        })
                    si["on_wait"] = waits[:1]
                out.append(ins)
            blk["instructions"] = out
    return bir


def build_kernel(dim=DIM, block=BLOCK, hq=HQ, hkv=HKV, mm_dt=MM_DT):
    """Emit the per-core Bass program (SPMD: same program on all cores)."""
    rep = hq // hkv
    KC = dim // 128          # contraction chunks
    assert KC % 2 == 0
    KH = KC // 2             # chunks per half
    W = min(512, block)      # psum free width
    CH = block // W          # i-chunks per block
    NI = block // 128        # j-tiles per block
    assert NI <= 8
    HSET = max(1, 8 // CH)   # q heads per psum-set
    HALF_D = D // 2
    SCALE = float(1.0 / np.sqrt(D))
    assert hkv * D <= 512

    nc = bass.Bass("TRN2", target_bir_lowering=False, debug=False)

    xbT = nc.dram_tensor("xbT", [dim, block], F32, kind="ExternalInput").ap()
    wq = nc.dram_tensor("wq", [dim, hq * D], F32, kind="ExternalInput").ap()
    wk = nc.dram_tensor("wk", [dim, hkv * D], F32, kind="ExternalInput").ap()
    wv = nc.dram_tensor("wv", [dim, hkv * D], F32, kind="ExternalInput").ap()
    wo = nc.dram_tensor("wo", [hq * D, dim], F32, kind="ExternalInput").ap()
    # cos2: [cos; cos] stacked to 128 partitions; sin2: [-sin; +sin]
    cos2 = nc.dram_tensor("cos2", [D, block], F32, kind="ExternalInput").ap()
    sin2 = nc.dram_tensor("sin2", [D, block], F32, kind="ExternalInput").ap()
    out = nc.dram_tensor("out", [block, dim], F32, kind="ExternalOutput").ap()

    MDT = mm_dt  # dtype of all matmul-input SBUF tiles

    def mm(out_ap, lhsT, rhs, **kw):
        nc.tensor.matmul(out_ap, lhsT, rhs, **kw)

    def ld(dst, src):
        # DMA load with dtype relabel for fp32r tiles
        if mm_dt != F32:
            src = src.bitcast(mm_dt)
        nc.sync.dma_start(dst, src)

    # expS free-dim layout: j-tile t occupies [offs[t], offs[t] + block - 128 t)
    offs = []
    o = 0
    for t in range(NI):
        offs.append(o)
        o += block - t * 128
    EW = o

    with tile.TileContext(nc) as tc, ExitStack() as ctx:
        const = ctx.enter_context(tc.tile_pool(name="const", bufs=1))
        ones = const.tile([128, 1], MDT)
        nc.gpsimd.memset(ones[:].bitcast(F32), 1.0)
        ones_row = const.tile([1, 128], MDT)
        nc.gpsimd.memset(ones_row[:].bitcast(F32), 1.0)
        # additive causal mask for the diagonal 128x128 block of S^T
        # (partition x = j_local, free y = i_local; invalid where y < x)
        tri = const.tile([128, 128], F32)
        nc.gpsimd.memset(tri[:], 0.0)
        # keep (0.0) where y - x >= 0 (i.e. i_local >= j_local), else NEG
        nc.gpsimd.affine_select(
            out=tri[:], in_=tri[:],
            compare_op=mybir.AluOpType.is_ge,
            fill=NEG, base=0, pattern=[[1, 128]], channel_multiplier=-1,
        )

        with tc.tile_pool(name="accs", bufs=1) as acc_pool:
            # persistent SBUF accumulators, one big tile each, sliced per head
            qTa = acc_pool.tile([128, hq * block], MDT)     # per head: [d, i]
            kTa = acc_pool.tile([128, hkv * block], MDT)    # per kv head: [d, j]
            va = acc_pool.tile([128, NI * hkv * D], MDT)    # per j-tile: [j, hkv*D]

            # ---- QKV projections, two k-halves so SBUF holds half of xbT.
            # ropep is allocated before xbp/wsp (fresh addresses, so the
            # rope swap DMAs carry no address-reuse waits) and released
            # with them, freeing its space for the attention pools. ----
            with tc.tile_pool(name="ropep", bufs=3) as rp, \
                 tc.tile_pool(name="xbp", bufs=KH) as xb_pool, \
                 tc.tile_pool(name="wsp", bufs=4) as ws_pool, \
                 tc.tile_pool(name="qkvps", bufs=8, space="PSUM") as ps_pool:

                # cos/sin live only until RoPE is done, so they share the
                # QKV-scoped pool space instead of the ctx-lifetime const pool
                cos_sb = rp.tile([D, block], F32, name="cos_sb", tag="cos", bufs=1)
                nc.sync.dma_start(cos_sb[:], cos2)
                sin_sb = rp.tile([D, block], F32, name="sin_sb", tag="sin", bufs=1)
                nc.sync.dma_start(sin_sb[:], sin2)

                def acc_store(dst, ps, half):
                    if half == 0:
                        nc.scalar.copy(dst, ps)
                    else:
                        nc.vector.tensor_add(dst, dst, ps)

                for half in range(2):
                    xb = []
                    for kk in range(KH):
                        k = half * KH + kk
                        xt = xb_pool.tile([128, block], MDT, name="xbt", tag="xb")
                        ld(xt[:], xbT[k * 128:(k + 1) * 128, :])
                        xb.append(xt)

                    # Q^T: per head [d, i] accumulated over k
                    for hs in range(0, hq, HSET):
                        nh = min(HSET, hq - hs)
                        ps = [[ps_pool.tile([128, W], F32, name="qps", tag="ps")
                               for _ in range(CH)] for _ in range(nh)]
                        for kk in range(KH):
                            k = half * KH + kk
                            wt = ws_pool.tile([128, HSET * D], MDT, name="wqs", tag="ws")
                            ld(wt[:, :nh * D],
                               wq[k * 128:(k + 1) * 128, hs * D:(hs + nh) * D])
                            for hl in range(nh):
                                for c in range(CH):
                                    mm(ps[hl][c][:], wt[:, hl * D:(hl + 1) * D],
                                       xb[kk][:, c * W:(c + 1) * W],
                                       start=(kk == 0), stop=(kk == KH - 1))
                        for hl in range(nh):
                            for c in range(CH):
                                h = hs + hl
                                dst = qTa[:, h * block + c * W: h * block + (c + 1) * W]
                                acc_store(dst, ps[hl][c][:], half)

                    # K^T: per kv head [d, j]
                    for hs in range(0, hkv, HSET):
                        nh = min(HSET, hkv - hs)
                        ps = [[ps_pool.tile([128, W], F32, name="kps", tag="ps")
                               for _ in range(CH)] for _ in range(nh)]
                        for kk in range(KH):
                            k = half * KH + kk
                            wt = ws_pool.tile([128, HSET * D], MDT, name="wks", tag="ws")
                            ld(wt[:, :nh * D],
                               wk[k * 128:(k + 1) * 128, hs * D:(hs + nh) * D])
                            for hl in range(nh):
                                for c in range(CH):
                                    mm(ps[hl][c][:], wt[:, hl * D:(hl + 1) * D],
                                       xb[kk][:, c * W:(c + 1) * W],
                                       start=(kk == 0), stop=(kk == KH - 1))
                        for hl in range(nh):
                            for c in range(CH):
                                h = hs + hl
                                dst = kTa[:, h * block + c * W: h * block + (c + 1) * W]
                                acc_store(dst, ps[hl][c][:], half)

                    # V: per j-tile [j, hkv*D]
                    for ts in range(0, NI, 8):
                        nt = min(8, NI - ts)
                        ps = [ps_pool.tile([128, W], F32, name="vps", tag="ps")
                              for _ in range(nt)]
                        for kk in range(KH):
                            k = half * KH + kk
                            wt = ws_pool.tile([128, HSET * D], MDT, name="wvs", tag="ws")
                            ld(wt[:, :hkv * D],
                               wv[k * 128:(k + 1) * 128, :])
                            for tl in range(nt):
                                tj = ts + tl
                                mm(ps[tl][:, :hkv * D],
                                   xb[kk][:, tj * 128:(tj + 1) * 128],
                                   wt[:, :hkv * D],
                                   start=(kk == 0), stop=(kk == KH - 1))
                        for tl in range(nt):
                            tj = ts + tl
                            dst = va[:, tj * hkv * D:(tj + 1) * hkv * D]
                            acc_store(dst, ps[tl][:, :hkv * D], half)

                # ---- RoPE on Q^T / K^T (in SBUF, in place) ----
                # Layout per head: partitions [0:64] = even head-dims,
                # [64:128] = odd.  rope(x) = x*cos2 + swap(x)*sin2 where
                # swap exchanges the halves (SBUF->SBUF DMA) and
                # sin2 = [-sin; +sin], cos2 = [cos; cos].
                def rope(base):
                    sw = rp.tile([D, block], MDT, name="sw", tag="sw")
                    nc.sync.dma_start(sw[0:HALF_D, :], base[HALF_D:D, :])
                    nc.sync.dma_start(sw[HALF_D:D, :], base[0:HALF_D, :])
                    tmp = rp.tile([D, block], F32, name="rtmp", tag="rtmp")
                    nc.vector.tensor_mul(tmp[:], sw[:], sin_sb[:])
                    nc.vector.tensor_mul(base, base, cos_sb[:])
                    nc.vector.tensor_add(base, base, tmp[:])

                for h in range(hq):
                    rope(qTa[:, h * block:(h + 1) * block])
                for h in range(hkv):
                    rope(kTa[:, h * block:(h + 1) * block])

            # O^T persists in SBUF through attention and the output
            # projection (no DRAM roundtrip); allocated after the QKV
            # pools release so it reuses their space, and entered on ctx
            # so it outlives the accs block.
            oT_pool = ctx.enter_context(
                tc.tile_pool(name="oTallp", bufs=1, side="right"))
            oTall = oT_pool.tile([128, hq * block], MDT, name="oTall")

            # ---- attention per q head ----
            with tc.tile_pool(name="attsb", bufs=2) as att_sb, \
                 tc.tile_pool(name="stps", bufs=3, space="PSUM") as st_ps, \
                 tc.tile_pool(name="sumps", bufs=2, space="PSUM") as sum_ps, \
                 tc.tile_pool(name="bcps", bufs=1, space="PSUM") as bc_ps, \
                 tc.tile_pool(name="pvps", bufs=2, space="PSUM") as pv_ps:
                def scores(h):
                    # S^T (causal trapezoid) -> exp
                    kv = h // rep
                    qT = qTa[:, h * block:(h + 1) * block]
                    kT = kTa[:, kv * block:(kv + 1) * block]
                    expS = att_sb.tile([128, EW], MDT, name="expS", tag="expS")
                    for t in range(NI):
                        i0 = t * 128
                        for c in range(CH):
                            s0 = max(i0, c * W)
                            s1 = (c + 1) * W
                            if s0 >= s1:
                                continue
                            w = s1 - s0
                            st = st_ps.tile([128, W], F32, name="st", tag="st")
                            mm(st[:, :w], kT[:, i0:i0 + 128], qT[:, s0:s1],
                               start=True, stop=True)
                            if s0 == i0:  # diagonal block at local [0:128)
                                nc.vector.tensor_add(st[:, 0:128], st[:, 0:128], tri[:])
                            e0 = offs[t] + (s0 - i0)
                            nc.scalar.activation(
                                expS[:, e0:e0 + w], st[:, :w],
                                mybir.ActivationFunctionType.Exp, scale=SCALE)
                    return expS

                def finish(h, expS):
                    # denominator (ones^T @ expS^T -> reciprocal), PV, then a
                    # K=1 outer-product broadcast of 1/sum and the normalize.
                    # PV is emitted before the broadcast so the PE never waits
                    # on the DVE reciprocal.
                    kv = h // rep
                    rcs = []
                    for c in range(CH):
                        live = [t for t in range(NI) if t * 128 < (c + 1) * W]
                        sp = sum_ps.tile([1, W], F32, name="sump", tag="sump")
                        for idx, t in enumerate(live):
                            i0 = t * 128
                            s0 = max(i0, c * W)
                            w = (c + 1) * W - s0
                            e0 = offs[t] + (s0 - i0)
                            mm(sp[:1, s0 - c * W:s0 - c * W + w],
                               ones[:, :1], expS[:, e0:e0 + w],
                               start=(idx == 0), stop=(idx == len(live) - 1))
                        rc = att_sb.tile([1, W], MDT, name="rc", tag="rc")
                        with nc.allow_low_precision("fp32r matmul operand"):
                            nc.vector.reciprocal(rc[:1, :W], sp[:1, :W])
                        rcs.append(rc)

                    recipB = att_sb.tile([128, block], F32, name="recipB",
                                         tag="recipB", bufs=1)
                    for c in range(CH):
                        bc = bc_ps.tile([128, W], F32, name="bc", tag="bc")
                        mm(bc[:], ones_row[:1, :], rcs[c][:1, :W],
                           start=True, stop=True)
                        nc.scalar.copy(recipB[:, c * W:(c + 1) * W], bc[:])

                    pvs = []
                    for c in range(CH):
                        live = [t for t in range(NI) if t * 128 < (c + 1) * W]
                        pv = pv_ps.tile([128, W], F32, name="pv", tag="pv")
                        for idx, t in enumerate(live):
                            i0 = t * 128
                            s0 = max(i0, c * W)
                            w = (c + 1) * W - s0
                            e0 = offs[t] + (s0 - i0)
                            mm(pv[:, s0 - c * W:s0 - c * W + w],
                               va[:, t * hkv * D + kv * D: t * hkv * D + (kv + 1) * D],
                               expS[:, e0:e0 + w],
                               start=(idx == 0), stop=(idx == len(live) - 1))
                        pvs.append(pv)

                    for c in range(CH):
                        nc.vector.tensor_mul(
                            oTall[:, h * block + c * W: h * block + (c + 1) * W],
                            pvs[c][:, :W],
                            recipB[:, c * W:(c + 1) * W])

                # two-stage pipeline: S^T/exp of head h+1 overlaps the
                # denominator/PV work of head h, so the PE always has
                # independent matmuls while ACT runs exp
                prev = None
                for h in range(hq):
                    e = scores(h)
                    if prev is not None:
                        finish(*prev)
                    prev = (h, e)
                finish(*prev)

        # ---- output projection: out = O @ wo_g ----
        with tc.tile_pool(name="wow", bufs=2 * hq) as wo_pool, \
             tc.tile_pool(name="woout", bufs=4) as out_pool, \
             tc.tile_pool(name="wops", bufs=4, space="PSUM") as wo_ps:
            for nch in range(dim // W):
                wts = []
                for h in range(hq):
                    wt = wo_pool.tile([128, W], MDT, name="wot", tag="wot")
                    ld(wt[:], wo[h * D:(h + 1) * D, nch * W:(nch + 1) * W])
                    wts.append(wt)
                for it in range(NI):
                    ps = wo_ps.tile([128, W], F32, name="wop", tag="wop")
                    for h in range(hq):
                        mm(ps[:], oTall[:, h * block + it * 128: h * block + it * 128 + 128],
                           wts[h][:], start=(h == 0), stop=(h == hq - 1))
                    ob = out_pool.tile([128, W], F32, name="ob", tag="ob")
                    nc.scalar.copy(ob[:], ps[:])
                    nc.sync.dma_start(out[it * 128:(it + 1) * 128, nch * W:(nch + 1) * W],
                                      ob[:])
    _trim_dma_waits(nc)
    import json as _json
    _fixed = _json.dumps(_split_waits_json(
        _json.loads(nc.to_json_bytes()))).encode()
    nc.to_json_bytes = lambda: _fixed
    return nc


def _deinterleave_cols(w, nheads):
    """Per head, reorder the 128 columns to [even head-dims, odd head-dims]."""
    dim = w.shape[0]
    r = w.reshape(dim, nheads, D // 2, 2)
    return np.concatenate([r[..., 0], r[..., 1]], axis=2).reshape(dim, nheads * D)


def shard_inputs(x, wq, wk, wv, wo, freqs_cos, freqs_sin):
    """Build the 8 per-core input maps (core = 2*block + head_group)."""
    x = np.ascontiguousarray(np.asarray(x, dtype=np.float32))
    wq_p = _deinterleave_cols(np.asarray(wq, dtype=np.float32), 32)
    wk_p = _deinterleave_cols(np.asarray(wk, dtype=np.float32), 8)
    wv = np.asarray(wv, dtype=np.float32)
    wo = np.asarray(wo, dtype=np.float32)
    cos = np.asarray(freqs_cos, dtype=np.float32)
    sin = np.asarray(freqs_sin, dtype=np.float32)

    wq_h = wq_p.reshape(DIM, 32, D)
    wk_h = wk_p.reshape(DIM, 8, D)
    wv_h = wv.reshape(DIM, 8, D)
    wo_h = wo.reshape(32, D, DIM)

    in_maps = []
    for core in range(N_CORES):
        b, g = divmod(core, 2)
        rows = slice(b * BLOCK, (b + 1) * BLOCK)
        cosT = cos[rows].T                       # [64, block]
        sinT = sin[rows].T
        cos2 = np.concatenate([cosT, cosT], axis=0)     # [128, block]
        sin2 = np.concatenate([-sinT, sinT], axis=0)
        in_maps.append({
            "xbT": np.ascontiguousarray(x[rows, :].T),
            "wq": np.ascontiguousarray(
                wq_h[:, g * HQ:(g + 1) * HQ].reshape(DIM, HQ * D)),
            "wk": np.ascontiguousarray(
                wk_h[:, g * HKV:(g + 1) * HKV].reshape(DIM, HKV * D)),
            "wv": np.ascontiguousarray(
                wv_h[:, g * HKV:(g + 1) * HKV].reshape(DIM, HKV * D)),
            "wo": np.ascontiguousarray(
                wo_h[g * HQ:(g + 1) * HQ].reshape(HQ * D, DIM)),
            "cos2": np.ascontiguousarray(cos2),
            "sin2": np.ascontiguousarray(sin2),
        })
    return in_maps


def unshard_output(core_outs):
    full = np.empty((NB_TOTAL, DIM), dtype=np.float32)
    for b in range(NB_TOTAL // BLOCK):
        full[b * BLOCK:(b + 1) * BLOCK] = core_outs[2 * b] + core_outs[2 * b + 1]
    return full


NB_TOTAL = 4096  # total sequence length

_NC_CACHE = {}


def _get_nc():
    key = (DIM, BLOCK, HQ, HKV, str(MM_DT))
    if key not in _NC_CACHE:
        _NC_CACHE[key] = build_kernel()
    return _NC_CACHE[key]


def kernel(x, wq, wk, wv, wo, freqs_cos, freqs_sin, block_size, **run_kwargs):
    assert int(block_size) == BLOCK, f"unexpected block_size {block_size}"
    in_maps = shard_inputs(x, wq, wk, wv, wo, freqs_cos, freqs_sin)
    nc = _get_nc()
    res = bass_utils.run_bass_kernel_spmd(
        nc, in_maps, core_ids=list(range(N_CORES)), **run_kwargs)
    outs = [r["out"] for r in res.results]
    out = unshard_output(outs)
    kernel.last_results = res
    return out


# revision 42
# speedup vs baseline: 1.0436x; 1.0170x over previous
"""Block-diagonal causal GQA attention with RoPE, sharded over 8 TRN2 cores.

Problem (hardcoded from the spec):
  x [4096, 4096], wq [4096, 4096] (32 q heads x 128), wk/wv [4096, 1024]
  (8 kv heads), wo [4096, 4096], freqs_cos/sin [4096, 64], block_size 1024.
  4 independent causal blocks of 1024 tokens.

Sharding: 8 cores = 4 sequence blocks x 2 head-groups.  Core (b, g)
computes block b for q-heads [16g, 16g+16) (kv heads [4g, 4g+4)) and the
partial output projection through the matching rows of wo.  The host sums
the two head-group partials per block and concatenates the blocks.

Device layout notes:
  - x block is fed pre-transposed (xbT [4096, 1024]) so the DIM contraction
    sits on SBUF partitions for the QKV projections.
  - wq/wk columns are de-interleaved per head on the host (even head-dims
    then odd head-dims) so RoPE's pair rotation becomes a [0:64]/[64:128]
    partition-half operation on the Q^T/K^T layout.
  - attention runs in the transposed-score layout S^T [j, i]: scores come
    out of the PE as S^T tiles, the softmax denominator is a ones-vector
    matmul, and P^T feeds the PV and WO matmuls directly (no transposes).
"""

import numpy as np
from contextlib import ExitStack

import concourse.bass as bass
import concourse.tile as tile
import concourse.mybir as mybir
from concourse import bass_utils
import concourse.tile_sem_assignment as _tsa

F32 = mybir.dt.float32

# -- full-problem constants ---------------------------------------------------
DIM = 4096
BLOCK = 1024
D = 128            # head dim
HQ = 16            # q heads per core
HKV = 4            # kv heads per core
N_CORES = 8
NEG = -1.0e9
# matmul compute dtype: float32 (4 cyc/row, accurate) or float32r (1 cyc/row)
import os as _os
MM_DT = (mybir.dt.float32r if _os.environ.get("MM_DT", "f32r") == "f32r"
         else mybir.dt.float32)


def _trim_dma_waits(nc):
    """Drop DMA semaphore waits that are transitively guaranteed.

    The DGE descriptor path supports only 2 sync-wait commands per DMA,
    but Tile's wait emission is not transitively minimal.  We compute,
    for every instruction, a conservative "floor": the semaphore values
    guaranteed to have been reached by the time it completes (its own
    waits, the floors of the instructions those waits observe, the
    floors of its sync dependencies, plus in-order completion along each
    semaphore's single FIFO ring).  A wait on a DMA is dead if the
    floors implied by its remaining waits already cover it.
    """
    import bass_rust

    insts = []
    for blk in nc.m.functions[0].blocks:
        insts.extend(blk.instructions)

    floors: dict[str, dict[int, int]] = {}     # inst name -> {sem id: value}
    chain: dict[int, list[tuple[int, str]]] = {}  # sem id -> [(post_val, name)]
    cum: dict[int, int] = {}

    def sem_floor(sem_id, v):
        """Floor implied by observing sem_id >= v (completion of the
        instruction whose update reached v, FIFO within a sem)."""
        lst = chain.get(sem_id)
        if not lst:
            return None
        # smallest post_val >= v
        import bisect
        idx = bisect.bisect_left(lst, (v, ""))
        if idx == len(lst):
            return None
        return floors.get(lst[idx][1])

    def merge(dst, src):
        if not src:
            return
        for k, v in src.items():
            if dst.get(k, -1) < v:
                dst[k] = v

    for ins in insts:
        si = ins.sync_info
        fl: dict[int, int] = {}
        if si is not None:
            for w in si.on_wait:
                if w.wait_mode != "sem-ge-imm" or w.wait_value is None:
                    continue
                if fl.get(w.id, -1) < w.wait_value:
                    fl[w.id] = w.wait_value
                merge(fl, sem_floor(w.id, w.wait_value))
        try:
            for dn in ins.sync_dependency_names():
                merge(fl, floors.get(dn))
        except TypeError:
            pass
        if si is not None:
            for u in si.on_update:
                if u.update_mode not in ("sem-add-imm", "sem-inc") \
                        or u.update_value is None:
                    continue
                post = cum.get(u.id, 0) + u.update_value
                cum[u.id] = post
                lst = chain.setdefault(u.id, [])
                # in-order completion per sem ring: inherit previous floor
                if lst:
                    merge(fl, floors.get(lst[-1][1]))
                if fl.get(u.id, -1) < post:
                    fl[u.id] = post
                lst.append((post, ins.name))
        floors[ins.name] = fl

    leftover = []
    for ins in insts:
        if not isinstance(ins, mybir.InstDMACopy):
            continue
        si = ins.sync_info
        if si is None:
            continue
        waits = list(si.on_wait)
        changed = True
        while len(waits) > 1 and changed:
            changed = False
            for i, w in enumerate(waits):
                if w.wait_mode != "sem-ge-imm" or w.wait_value is None:
                    continue
                implied: dict[int, int] = {}
                for j, w2 in enumerate(waits):
                    if j == i or w2.wait_mode != "sem-ge-imm":
                        continue
                    merge(implied, sem_floor(w2.id, w2.wait_value))
                if implied.get(w.id, -1) >= w.wait_value:
                    waits.pop(i)
                    changed = True
                    break
        if len(waits) != len(si.on_wait):
            ins.sync_info = bass_rust.SyncInfo(
                on_wait=waits, on_update=list(si.on_update))
    del leftover  # remaining multi-waits are split into engine prefixes


def _split_waits_json(bir):
    """Split multi-wait instructions at the BIR level.

    walrus' setupSyncWait budget: one wait of any value, or two waits
    whose values both fit a one-byte command.  Excess waits move onto
    standalone EventSemaphore instructions inserted directly before the
    instruction on the same engine — engines execute their stream in
    order, so a prefix wait is semantically identical to an attached
    one.  (DMAs are enqueued by their issuing engine in stream order,
    so the same argument holds for the enqueue.)
    """
    nid = 0
    for fn in bir["functions"]:
        for blk in fn["blocks"]:
            out = []
            for ins in blk["instructions"]:
                si = ins.get("sync_info")
                waits = (si or {}).get("on_wait") or []
                if len(waits) > 1:
                    waits = sorted(
                        waits, key=lambda w: -(w.get("wait_value") or 0))
                    for w in waits[1:]:
                        nid += 1
                        out.append({
                            "debug": ins.get("debug"),
                            "engine": ins["engine"],
                            "ins": [],
                            "outs": [],
                            "name": f"{ins['name']}-w{nid}",
                            "opcode": "EventSemaphore",
                            "sync_info": {"on_update": [], "on_wait": [w]},
                        })
                    si["on_wait"] = waits[:1]
                out.append(ins)
            blk["instructions"] = out
    return bir


def build_kernel(dim=DIM, block=BLOCK, hq=HQ, hkv=HKV, mm_dt=MM_DT):
    """Emit the per-core Bass program (SPMD: same program on all cores)."""
    rep = hq // hkv
    KC = dim // 128          # contraction chunks
    assert KC % 2 == 0
    KH = KC // 2             # chunks per half
    W = min(512, block)      # psum free width
    CH = block // W          # i-chunks per block
    NI = block // 128        # j-tiles per block
    assert NI <= 8
    HSET = max(1, 8 // CH)   # q heads per psum-set
    HALF_D = D // 2
    SCALE = float(1.0 / np.sqrt(D))
    assert hkv * D <= 512

    nc = bass.Bass("TRN2", target_bir_lowering=False, debug=False)

    xbT = nc.dram_tensor("xbT", [dim, block], F32, kind="ExternalInput").ap()
    wq = nc.dram_tensor("wq", [dim, hq * D], F32, kind="ExternalInput").ap()
    wk = nc.dram_tensor("wk", [dim, hkv * D], F32, kind="ExternalInput").ap()
    wv = nc.dram_tensor("wv", [dim, hkv * D], F32, kind="ExternalInput").ap()
    wo = nc.dram_tensor("wo", [hq * D, dim], F32, kind="ExternalInput").ap()
    # cos2: [cos; cos] stacked to 128 partitions; sin2: [-sin; +sin]
    cos2 = nc.dram_tensor("cos2", [D, block], F32, kind="ExternalInput").ap()
    sin2 = nc.dram_tensor("sin2", [D, block], F32, kind="ExternalInput").ap()
    out = nc.dram_tensor("out", [block, dim], F32, kind="ExternalOutput").ap()

    MDT = mm_dt  # dtype of all matmul-input SBUF tiles

    def mm(out_ap, lhsT, rhs, **kw):
        nc.tensor.matmul(out_ap, lhsT, rhs, **kw)

    def ld(dst, src):
        # DMA load with dtype relabel for fp32r tiles
        if mm_dt != F32:
            src = src.bitcast(mm_dt)
        nc.sync.dma_start(dst, src)

    # expS free-dim layout: j-tile t occupies [offs[t], offs[t] + block - 128 t)
    offs = []
    o = 0
    for t in range(NI):
        offs.append(o)
        o += block - t * 128
    EW = o

    with tile.TileContext(nc) as tc, ExitStack() as ctx:
        const = ctx.enter_context(tc.tile_pool(name="const", bufs=1))
        ones = const.tile([128, 1], MDT)
        nc.gpsimd.memset(ones[:].bitcast(F32), 1.0)
        ones_row = const.tile([1, 128], MDT)
        nc.gpsimd.memset(ones_row[:].bitcast(F32), 1.0)
        # additive causal mask for the diagonal 128x128 block of S^T
        # (partition x = j_local, free y = i_local; invalid where y < x)
        tri = const.tile([128, 128], F32)
        nc.gpsimd.memset(tri[:], 0.0)
        # keep (0.0) where y - x >= 0 (i.e. i_local >= j_local), else NEG
        nc.gpsimd.affine_select(
            out=tri[:], in_=tri[:],
            compare_op=mybir.AluOpType.is_ge,
            fill=NEG, base=0, pattern=[[1, 128]], channel_multiplier=-1,
        )

        with tc.tile_pool(name="accs", bufs=1) as acc_pool:
            # persistent SBUF accumulators, one big tile each, sliced per head
            qTa = acc_pool.tile([128, hq * block], MDT)     # per head: [d, i]
            kTa = acc_pool.tile([128, hkv * block], MDT)    # per kv head: [d, j]
            va = acc_pool.tile([128, NI * hkv * D], MDT)    # per j-tile: [j, hkv*D]

            # ---- QKV projections, two k-halves so SBUF holds half of xbT.
            # ropep is allocated before xbp/wsp (fresh addresses, so the
            # rope swap DMAs carry no address-reuse waits) and released
            # with them, freeing its space for the attention pools. ----
            with tc.tile_pool(name="ropep", bufs=3) as rp, \
                 tc.tile_pool(name="xbp", bufs=KH) as xb_pool, \
                 tc.tile_pool(name="wsp", bufs=4) as ws_pool, \
                 tc.tile_pool(name="qkvps", bufs=8, space="PSUM") as ps_pool:

                # cos/sin live only until RoPE is done, so they share the
                # QKV-scoped pool space instead of the ctx-lifetime const pool
                cos_sb = rp.tile([D, block], F32, name="cos_sb", tag="cos", bufs=1)
                nc.sync.dma_start(cos_sb[:], cos2)
                sin_sb = rp.tile([D, block], F32, name="sin_sb", tag="sin", bufs=1)
                nc.sync.dma_start(sin_sb[:], sin2)

                def acc_store(dst, ps, half):
                    if half == 0:
                        nc.scalar.copy(dst, ps)
                    else:
                        nc.vector.tensor_add(dst, dst, ps)

                for half in range(2):
                    # xb loads are interleaved into head-set 0's kk loop:
                    # the DGE ring is FIFO, so bulk-loading all of xb first
                    # would stall the first weight tile (and the first
                    # matmul) behind 8 MB of transfers.
                    xb = []

                    # Q^T: per head [d, i] accumulated over k
                    for hs in range(0, hq, HSET):
                        nh = min(HSET, hq - hs)
                        ps = [[ps_pool.tile([128, W], F32, name="qps", tag="ps")
                               for _ in range(CH)] for _ in range(nh)]
                        for kk in range(KH):
                            k = half * KH + kk
                            if hs == 0:
                                xt = xb_pool.tile([128, block], MDT,
                                                  name="xbt", tag="xb")
                                ld(xt[:], xbT[k * 128:(k + 1) * 128, :])
                                xb.append(xt)
                            wt = ws_pool.tile([128, HSET * D], MDT, name="wqs", tag="ws")
                            ld(wt[:, :nh * D],
                               wq[k * 128:(k + 1) * 128, hs * D:(hs + nh) * D])
                            for hl in range(nh):
                                for c in range(CH):
                                    mm(ps[hl][c][:], wt[:, hl * D:(hl + 1) * D],
                                       xb[kk][:, c * W:(c + 1) * W],
                                       start=(kk == 0), stop=(kk == KH - 1))
                        for hl in range(nh):
                            for c in range(CH):
                                h = hs + hl
                                dst = qTa[:, h * block + c * W: h * block + (c + 1) * W]
                                acc_store(dst, ps[hl][c][:], half)

                    # K^T: per kv head [d, j]
                    for hs in range(0, hkv, HSET):
                        nh = min(HSET, hkv - hs)
                        ps = [[ps_pool.tile([128, W], F32, name="kps", tag="ps")
                               for _ in range(CH)] for _ in range(nh)]
                        for kk in range(KH):
                            k = half * KH + kk
                            wt = ws_pool.tile([128, HSET * D], MDT, name="wks", tag="ws")
                            ld(wt[:, :nh * D],
                               wk[k * 128:(k + 1) * 128, hs * D:(hs + nh) * D])
                            for hl in range(nh):
                                for c in range(CH):
                                    mm(ps[hl][c][:], wt[:, hl * D:(hl + 1) * D],
                                       xb[kk][:, c * W:(c + 1) * W],
                                       start=(kk == 0), stop=(kk == KH - 1))
                        for hl in range(nh):
                            for c in range(CH):
                                h = hs + hl
                                dst = kTa[:, h * block + c * W: h * block + (c + 1) * W]
                                acc_store(dst, ps[hl][c][:], half)

                    # V: per j-tile [j, hkv*D]
                    for ts in range(0, NI, 8):
                        nt = min(8, NI - ts)
                        ps = [ps_pool.tile([128, W], F32, name="vps", tag="ps")
                              for _ in range(nt)]
                        for kk in range(KH):
                            k = half * KH + kk
                            wt = ws_pool.tile([128, HSET * D], MDT, name="wvs", tag="ws")
                            ld(wt[:, :hkv * D],
                               wv[k * 128:(k + 1) * 128, :])
                            for tl in range(nt):
                                tj = ts + tl
                                mm(ps[tl][:, :hkv * D],
                                   xb[kk][:, tj * 128:(tj + 1) * 128],
                                   wt[:, :hkv * D],
                                   start=(kk == 0), stop=(kk == KH - 1))
                        for tl in range(nt):
                            tj = ts + tl
                            dst = va[:, tj * hkv * D:(tj + 1) * hkv * D]
                            acc_store(dst, ps[tl][:, :hkv * D], half)

                # ---- RoPE on Q^T / K^T (in SBUF, in place) ----
                # Layout per head: partitions [0:64] = even head-dims,
                # [64:128] = odd.  rope(x) = x*cos2 + swap(x)*sin2 where
                # swap exchanges the halves (SBUF->SBUF DMA) and
                # sin2 = [-sin; +sin], cos2 = [cos; cos].
                def rope(base):
                    sw = rp.tile([D, block], MDT, name="sw", tag="sw")
                    nc.sync.dma_start(sw[0:HALF_D, :], base[HALF_D:D, :])
                    nc.sync.dma_start(sw[HALF_D:D, :], base[0:HALF_D, :])
                    tmp = rp.tile([D, block], F32, name="rtmp", tag="rtmp")
                    nc.vector.tensor_mul(tmp[:], sw[:], sin_sb[:])
                    nc.vector.tensor_mul(base, base, cos_sb[:])
                    nc.vector.tensor_add(base, base, tmp[:])

                for h in range(hq):
                    rope(qTa[:, h * block:(h + 1) * block])
                for h in range(hkv):
                    rope(kTa[:, h * block:(h + 1) * block])

            # O^T persists in SBUF through attention and the output
            # projection (no DRAM roundtrip); allocated after the QKV
            # pools release so it reuses their space, and entered on ctx
            # so it outlives the accs block.
            oT_pool = ctx.enter_context(
                tc.tile_pool(name="oTallp", bufs=1, side="right"))
            oTall = oT_pool.tile([128, hq * block], MDT, name="oTall")

            # ---- attention per q head ----
            with tc.tile_pool(name="attsb", bufs=2) as att_sb, \
                 tc.tile_pool(name="stps", bufs=3, space="PSUM") as st_ps, \
                 tc.tile_pool(name="sumps", bufs=2, space="PSUM") as sum_ps, \
                 tc.tile_pool(name="bcps", bufs=1, space="PSUM") as bc_ps, \
                 tc.tile_pool(name="pvps", bufs=2, space="PSUM") as pv_ps:
                def scores(h):
                    # S^T (causal trapezoid) -> exp
                    kv = h // rep
                    qT = qTa[:, h * block:(h + 1) * block]
                    kT = kTa[:, kv * block:(kv + 1) * block]
                    expS = att_sb.tile([128, EW], MDT, name="expS", tag="expS")
                    for t in range(NI):
                        i0 = t * 128
                        for c in range(CH):
                            s0 = max(i0, c * W)
                            s1 = (c + 1) * W
                            if s0 >= s1:
                                continue
                            w = s1 - s0
                            st = st_ps.tile([128, W], F32, name="st", tag="st")
                            mm(st[:, :w], kT[:, i0:i0 + 128], qT[:, s0:s1],
                               start=True, stop=True)
                            if s0 == i0:  # diagonal block at local [0:128)
                                nc.vector.tensor_add(st[:, 0:128], st[:, 0:128], tri[:])
                            e0 = offs[t] + (s0 - i0)
                            nc.scalar.activation(
                                expS[:, e0:e0 + w], st[:, :w],
                                mybir.ActivationFunctionType.Exp, scale=SCALE)
                    return expS

                def finish(h, expS):
                    # denominator (ones^T @ expS^T -> reciprocal), PV, then a
                    # K=1 outer-product broadcast of 1/sum and the normalize.
                    # PV is emitted before the broadcast so the PE never waits
                    # on the DVE reciprocal.
                    kv = h // rep
                    rcs = []
                    for c in range(CH):
                        live = [t for t in range(NI) if t * 128 < (c + 1) * W]
                        sp = sum_ps.tile([1, W], F32, name="sump", tag="sump")
                        for idx, t in enumerate(live):
                            i0 = t * 128
                            s0 = max(i0, c * W)
                            w = (c + 1) * W - s0
                            e0 = offs[t] + (s0 - i0)
                            mm(sp[:1, s0 - c * W:s0 - c * W + w],
                               ones[:, :1], expS[:, e0:e0 + w],
                               start=(idx == 0), stop=(idx == len(live) - 1))
                        rc = att_sb.tile([1, W], MDT, name="rc", tag="rc")
                        with nc.allow_low_precision("fp32r matmul operand"):
                            nc.vector.reciprocal(rc[:1, :W], sp[:1, :W])
                        rcs.append(rc)

                    recipB = att_sb.tile([128, block], F32, name="recipB",
                                         tag="recipB", bufs=1)
                    for c in range(CH):
                        bc = bc_ps.tile([128, W], F32, name="bc", tag="bc")
                        mm(bc[:], ones_row[:1, :], rcs[c][:1, :W],
                           start=True, stop=True)
                        nc.scalar.copy(recipB[:, c * W:(c + 1) * W], bc[:])

                    pvs = []
                    for c in range(CH):
                        live = [t for t in range(NI) if t * 128 < (c + 1) * W]
                        pv = pv_ps.tile([128, W], F32, name="pv", tag="pv")
                        for idx, t in enumerate(live):
                            i0 = t * 128
                            s0 = max(i0, c * W)
                            w = (c + 1) * W - s0
                            e0 = offs[t] + (s0 - i0)
                            mm(pv[:, s0 - c * W:s0 - c * W + w],
                               va[:, t * hkv * D + kv * D: t * hkv * D + (kv + 1) * D],
                               expS[:, e0:e0 + w],
                               start=(idx == 0), stop=(idx == len(live) - 1))
                        pvs.append(pv)

                    for c in range(CH):
                        nc.vector.tensor_mul(
                            oTall[:, h * block + c * W: h * block + (c + 1) * W],
                            pvs[c][:, :W],
                            recipB[:, c * W:(c + 1) * W])

                # two-stage pipeline: S^T/exp of head h+1 overlaps the
                # denominator/PV work of head h, so the PE always has
                # independent matmuls while ACT runs exp
                prev = None
                for h in range(hq):
                    e = scores(h)
                    if prev is not None:
                        finish(*prev)
                    prev = (h, e)
                finish(*prev)

        # ---- output projection: out = O @ wo_g ----
        with tc.tile_pool(name="wow", bufs=2 * hq) as wo_pool, \
             tc.tile_pool(name="woout", bufs=4) as out_pool, \
             tc.tile_pool(name="wops", bufs=4, space="PSUM") as wo_ps:
            for nch in range(dim // W):
                wts = []
                for h in range(hq):
                    wt = wo_pool.tile([128, W], MDT, name="wot", tag="wot")
                    ld(wt[:], wo[h * D:(h + 1) * D, nch * W:(nch + 1) * W])
                    wts.append(wt)
                for it in range(NI):
                    ps = wo_ps.tile([128, W], F32, name="wop", tag="wop")
                    for h in range(hq):
                        mm(ps[:], oTall[:, h * block + it * 128: h * block + it * 128 + 128],
                           wts[h][:], start=(h == 0), stop=(h == hq - 1))
                    ob = out_pool.tile([128, W], F32, name="ob", tag="ob")
                    nc.scalar.copy(ob[:], ps[:])
                    nc.sync.dma_start(out[it * 128:(it + 1) * 128, nch * W:(nch + 1) * W],
                                      ob[:])
    _trim_dma_waits(nc)
    import json as _json
    _fixed = _json.dumps(_split_waits_json(
        _json.loads(nc.to_json_bytes()))).encode()
    nc.to_json_bytes = lambda: _fixed
    return nc


def _deinterleave_cols(w, nheads):
    """Per head, reorder the 128 columns to [even head-dims, odd head-dims]."""
    dim = w.shape[0]
    r = w.reshape(dim, nheads, D // 2, 2)
    return np.concatenate([r[..., 0], r[..., 1]], axis=2).reshape(dim, nheads * D)


def shard_inputs(x, wq, wk, wv, wo, freqs_cos, freqs_sin):
    """Build the 8 per-core input maps (core = 2*block + head_group)."""
    x = np.ascontiguousarray(np.asarray(x, dtype=np.float32))
    wq_p = _deinterleave_cols(np.asarray(wq, dtype=np.float32), 32)
    wk_p = _deinterleave_cols(np.asarray(wk, dtype=np.float32), 8)
    wv = np.asarray(wv, dtype=np.float32)
    wo = np.asarray(wo, dtype=np.float32)
    cos = np.asarray(freqs_cos, dtype=np.float32)
    sin = np.asarray(freqs_sin, dtype=np.float32)

    wq_h = wq_p.reshape(DIM, 32, D)
    wk_h = wk_p.reshape(DIM, 8, D)
    wv_h = wv.reshape(DIM, 8, D)
    wo_h = wo.reshape(32, D, DIM)

    in_maps = []
    for core in range(N_CORES):
        b, g = divmod(core, 2)
        rows = slice(b * BLOCK, (b + 1) * BLOCK)
        cosT = cos[rows].T                       # [64, block]
        sinT = sin[rows].T
        cos2 = np.concatenate([cosT, cosT], axis=0)     # [128, block]
        sin2 = np.concatenate([-sinT, sinT], axis=0)
        in_maps.append({
            "xbT": np.ascontiguousarray(x[rows, :].T),
            "wq": np.ascontiguousarray(
                wq_h[:, g * HQ:(g + 1) * HQ].reshape(DIM, HQ * D)),
            "wk": np.ascontiguousarray(
                wk_h[:, g * HKV:(g + 1) * HKV].reshape(DIM, HKV * D)),
            "wv": np.ascontiguousarray(
                wv_h[:, g * HKV:(g + 1) * HKV].reshape(DIM, HKV * D)),
            "wo": np.ascontiguousarray(
                wo_h[g * HQ:(g + 1) * HQ].reshape(HQ * D, DIM)),
            "cos2": np.ascontiguousarray(cos2),
            "sin2": np.ascontiguousarray(sin2),
        })
    return in_maps


def unshard_output(core_outs):
    full = np.empty((NB_TOTAL, DIM), dtype=np.float32)
    for b in range(NB_TOTAL // BLOCK):
        full[b * BLOCK:(b + 1) * BLOCK] = core_outs[2 * b] + core_outs[2 * b + 1]
    return full


NB_TOTAL = 4096  # total sequence length

_NC_CACHE = {}


def _get_nc():
    key = (DIM, BLOCK, HQ, HKV, str(MM_DT))
    if key not in _NC_CACHE:
        _NC_CACHE[key] = build_kernel()
    return _NC_CACHE[key]


def kernel(x, wq, wk, wv, wo, freqs_cos, freqs_sin, block_size, **run_kwargs):
    assert int(block_size) == BLOCK, f"unexpected block_size {block_size}"
    in_maps = shard_inputs(x, wq, wk, wv, wo, freqs_cos, freqs_sin)
    nc = _get_nc()
    res = bass_utils.run_bass_kernel_spmd(
        nc, in_maps, core_ids=list(range(N_CORES)), **run_kwargs)
    outs = [r["out"] for r in res.results]
    out = unshard_output(outs)
    kernel.last_results = res
    return out
